# revision 1
# baseline (speedup 1.0000x reference)
"""Local-window (banded) multi-head attention on 8 Trainium2 NeuronCores.

Problem: x[L=2048, B=8, D=512], Wqkv[1536, 512], Wout[512, 512], bout[512].
  qkv = x @ Wqkv.T ; per-head banded attention (|i-j| <= 64, window 129);
  out = attn_out @ Wout.T + bout.

Sharding: batch B=8 across the 8 cores (data parallel). Each core runs the
full pipeline for one batch element. Inputs are pre-transposed host-side so
all device matmuls contract over the partition dimension:

  xT[d, l], WqkvT[d, c], WoutT[d', c] in SBUF; scores are computed
  TRANSPOSED (scoresT[m, l] = K @ Q^T) so that softmax normalization and
  the P@V contraction both happen along the partition (m) axis with zero
  on-chip transposes. The softmax denominator comes for free from an
  appended ones-column in V; normalization uses a tiny K=1 broadcast matmul.

Matmuls run in float32r (fp32 storage, fast PE path, N>=256).
"""

import os
import sys

import numpy as np

if "/opt/trn_rl_repo" not in sys.path:
    sys.path.insert(0, "/opt/trn_rl_repo")

L, B, D, H, DH = 2048, 8, 512, 8, 64
WIN, PAD = 129, 64
C3 = 3 * D  # 1536
NK = D // 128  # 4 contraction tiles
NLT = L // 128  # 16 l-tiles
NCH = L // 256  # 8 attention l-chunks of 256
HEAD_STRIDE = DH + 1  # 65: V columns per head incl. ones column

_NC_CACHE = {}


def _build_nc():
    from concourse import bacc, mybir, tile

    f32 = mybir.dt.float32
    f32r = mybir.dt.float32r
    Exp = mybir.ActivationFunctionType.Exp
    is_ge = mybir.AluOpType.is_ge

    nc = bacc.Bacc(None, target_bir_lowering=False)

    xT_d = nc.dram_tensor("xT", [D, L], f32r, kind="ExternalInput")
    wqkvT_d = nc.dram_tensor("wqkvT", [D, C3], f32r, kind="ExternalInput")
    woutT_d = nc.dram_tensor("woutT", [D, D], f32r, kind="ExternalInput")
    bout_d = nc.dram_tensor("bout", [D], f32, kind="ExternalInput")
    zeros_d = nc.dram_tensor("zeros_c", [128, 640], f32r, kind="ExternalInput")
    onesc_d = nc.dram_tensor("ones_c", [128, 8], f32r, kind="ExternalInput")
    y_d = nc.dram_tensor("y", [L, D], f32, kind="ExternalOutput")

    import concourse.bass as bass

    KTW = 64 + L + 64  # K^T cols: zero-pad both sides

    with tile.TileContext(nc) as tc, nc.allow_low_precision(
        reason="float32r tiles feed the PE fast path; accumulation stays fp32 in PSUM"
    ):
        with (
            tc.tile_pool(name="pers", bufs=1) as pers,
            tc.tile_pool(name="ps", bufs=1, space="PSUM") as ps,
        ):
            # ---- persistent SBUF tensors (everything stays resident) ----
            xT = [pers.tile([128, L], f32r, name=f"xT{k}", tag=f"xT{k}") for k in range(NK)]
            wqkvT = [
                pers.tile([128, C3], f32r, name=f"wqkvT{k}", tag=f"wqkvT{k}")
                for k in range(NK)
            ]
            woutT = [
                pers.tile([128, D], f32r, name=f"woutT{k}", tag=f"woutT{k}")
                for k in range(NK)
            ]
            boutb = pers.tile([128, D], f32, name="boutb", tag="boutb")
            ones1 = pers.tile([1, DH], f32r, name="ones1", tag="ones1")
            QT = [pers.tile([128, L], f32r, name=f"QT{t}", tag=f"QT{t}") for t in range(NK)]
            KT = [
                pers.tile([128, KTW], f32r, name=f"KT{t}", tag=f"KT{t}")
                for t in range(NK)
            ]
            Vs = [
                pers.tile([128, H * HEAD_STRIDE], f32r, name=f"Vs{j}", tag=f"Vs{j}")
                for j in range(NLT + 1)
            ]
            # per-chunk normalized O^T buffers come from a rotating pool
            # (allocated per (t, ch) inside the loop)

            def mm(out, lhsT, rhs, start, stop):
                nc.tensor.matmul(out, lhsT, rhs, start=start, stop=stop)

            # ---- input DMAs: column-sliced + interleaved across both HWDGE
            # rings so the first projection groups unblock within ~3us ----
            for ch in range(4):
                cs = slice(ch * 512, (ch + 1) * 512)
                for k in range(NK):
                    eng = nc.sync if (k + ch) % 2 == 0 else nc.scalar
                    eng.dma_start(out=xT[k][:, cs], in_=xT_d[k * 128 : (k + 1) * 128, cs])
                # wqkvT thirds in Q, K, V priority order per round
                third = [0, D, 2 * D, None][ch]
                if third is not None:
                    ws = slice(third, third + 512)
                    for k in range(NK):
                        eng = nc.scalar if (k + ch) % 2 == 0 else nc.sync
                        eng.dma_start(
                            out=wqkvT[k][:, ws], in_=wqkvT_d[k * 128 : (k + 1) * 128, ws]
                        )
            for k in range(NK):
                nc.sync.dma_start(
                    out=woutT[k][:], in_=woutT_d[k * 128 : (k + 1) * 128, :]
                )
            bout_ap = bout_d[:]
            bout_bcast = bass.AP(
                tensor=bout_ap.tensor, offset=bout_ap.offset, ap=[[0, 128], [1, D]]
            )
            nc.gpsimd.dma_start(out=boutb[:], in_=bout_bcast)
            nc.gpsimd.dma_start(
                out=ones1[:], in_=onesc_d[0:DH, 0:1].rearrange("a b -> b a")
            )
            # zero K^T left pad and the out-of-range halves of the shifted V
            for t in range(NK):
                nc.sync.dma_start(out=KT[t][:, 0:64], in_=zeros_d[:, 0:64])
                nc.sync.dma_start(
                    out=KT[t][:, 64 + L : KTW], in_=zeros_d[:, 0:64]
                )
            nc.sync.dma_start(
                out=Vs[0][0:64, :], in_=zeros_d[0:64, 0 : H * HEAD_STRIDE]
            )
            nc.sync.dma_start(
                out=Vs[NLT][64:128, :], in_=zeros_d[0:64, 0 : H * HEAD_STRIDE]
            )
            # ones column for every head slot (softmax denom via PV matmul)
            for j in range(NLT + 1):
                vcol = Vs[j].rearrange("p (h e) -> p h e", e=HEAD_STRIDE)
                nc.gpsimd.dma_start(
                    out=vcol[:, :, DH : DH + 1],
                    in_=onesc_d[:].rearrange("p (h e) -> p h e", e=1),
                )

            # ---- phase B: projections, interleaved so attention unblocks
            # early: Q/K chunk round first, then a slice of V tiles ----
            def b1_vproj(lts):
                for lt in lts:
                    vp = ps.tile([128, D], f32, name=f"vp{lt}", tag="big", bufs=2)
                    for k in range(NK):
                        mm(
                            vp[:],
                            xT[k][:, lt * 128 : (lt + 1) * 128],
                            wqkvT[k][:, 2 * D : 3 * D],
                            start=(k == 0),
                            stop=(k == NK - 1),
                        )
                    src_v = vp.rearrange("p (h e) -> p h e", e=DH)
                    dlo = Vs[lt][64:128, :].rearrange("p (h e) -> p h e", e=HEAD_STRIDE)
                    dhi = Vs[lt + 1][0:64, :].rearrange(
                        "p (h e) -> p h e", e=HEAD_STRIDE
                    )
                    nc.scalar.copy(out=dlo[:, :, 0:DH], in_=src_v[0:64])
                    nc.vector.tensor_copy(out=dhi[:, :, 0:DH], in_=src_v[64:128])

            for ch in range(4):  # l-chunks of 512
                for t in range(NK):
                    for which in range(2):  # 0 -> Q tile t, 1 -> K tile t
                        c0 = which * D + t * 128
                        qp = ps.tile(
                            [128, 512], f32, name=f"qp{t}_{which}_{ch}",
                            tag="big", bufs=2,
                        )
                        for k in range(NK):
                            mm(
                                qp[:],
                                wqkvT[k][:, c0 : c0 + 128],
                                xT[k][:, ch * 512 : (ch + 1) * 512],
                                start=(k == 0),
                                stop=(k == NK - 1),
                            )
                        if which == 0:
                            dest = QT[t][:, ch * 512 : (ch + 1) * 512]
                        else:
                            dest = KT[t][:, 64 + ch * 512 : 64 + (ch + 1) * 512]
                        nc.vector.tensor_copy(out=dest, in_=qp[:])
                b1_vproj(range(4 * ch, 4 * ch + 4))

            # ---- phase C+D: banded attention + fused output projection ----
            def emit_D(dch, bufs):
                # output projection for chunk dch's two l-tiles
                for half in range(2):
                    lt = 2 * dch + half
                    yp = ps.tile([128, D], f32, name=f"yp{lt}", tag="big", bufs=2)
                    for k in range(NK):
                        mm(
                            yp[:],
                            bufs[k][:, half * 128 : (half + 1) * 128],
                            woutT[k][:],
                            start=(k == 0),
                            stop=(k == NK - 1),
                        )
                    ysb = pers.tile([128, D], f32, name=f"ysb{lt}", tag="ysb", bufs=2)
                    nc.vector.tensor_add(out=ysb[:], in0=yp[:], in1=boutb[:])
                    nc.sync.dma_start(out=y_d[lt * 128 : (lt + 1) * 128, :], in_=ysb[:])

            prev_otc = None
            for ch in range(NCH):
                cur_otc = []
                for t in range(NK):
                    if t == 2 and prev_otc is not None:
                        emit_D(ch - 1, prev_otc)
                    otc = None
                    otmp = None
                    for hh in range(2):
                        h = 2 * t + hh
                        p0 = hh * 64
                        qsl = QT[t][p0 : p0 + 64, ch * 256 : (ch + 1) * 256]
                        # fused scores psum: 3 m-tiles side by side (2 banks)
                        scp = ps.tile(
                            [128, 768], f32, name=f"sc{h}_{ch}", tag="sc", bufs=2
                        )
                        for r in range(3):
                            kcol = 256 * ch + 128 * r  # into padded KT columns
                            mm(
                                scp[:, 256 * r : 256 * (r + 1)],
                                KT[t][p0 : p0 + 64, kcol : kcol + 128],
                                qsl,
                                start=True,
                                stop=True,
                            )
                        pt = wk_tile = pers.tile(
                            [128, 768], f32r, name=f"pt{h}_{ch}", tag="pt", bufs=4
                        )
                        nc.scalar.activation(
                            out=pt[:], in_=scp[:], func=Exp, scale=0.125
                        )
                        # band mask per m-tile r: keep iff 0 <= (128r + p) - f <= 128
                        # fused as two 2-block selects over the 768-wide tile
                        pAB = pt[:, 0:512].rearrange("p (b f) -> p b f", f=256)
                        pBC = pt[:, 256:768].rearrange("p (b f) -> p b f", f=256)
                        nc.gpsimd.affine_select(
                            out=pAB, in_=pAB, compare_op=is_ge, fill=0.0,
                            base=0, pattern=[[128, 2], [-1, 256]],
                            channel_multiplier=1,
                        )
                        nc.gpsimd.affine_select(
                            out=pBC, in_=pBC, compare_op=is_ge, fill=0.0,
                            base=0, pattern=[[-128, 2], [1, 256]],
                            channel_multiplier=-1,
                        )
                        if ch == 0:  # global key index p-64 must be >= 0 (r0)
                            p_r0 = pt[:, 0:256]
                            nc.gpsimd.affine_select(
                                out=p_r0, in_=p_r0, compare_op=is_ge, fill=0.0,
                                base=-64, pattern=[[0, 256]], channel_multiplier=1,
                            )
                        if ch == NCH - 1:  # global key index 1984+p < L (r2)
                            p_r2 = pt[:, 512:768]
                            nc.gpsimd.affine_select(
                                out=p_r2, in_=p_r2, compare_op=is_ge, fill=0.0,
                                base=63, pattern=[[0, 256]], channel_multiplier=-1,
                            )
                        # P~ @ V (transposed): O'[d, l] with denom in row DH.
                        # Both heads share one PSUM bank (disjoint column halves;
                        # PE executes matmuls in program order, so hh=1's
                        # start=True bank-clear cannot interleave hh=0's group).
                        if hh == 0:
                            op = ps.tile(
                                [DH + 1, 512], f32, name=f"op{t}_{ch}", tag="o",
                                bufs=2,
                            )
                        for r in range(3):
                            vsl = Vs[2 * ch + r][
                                :, h * HEAD_STRIDE : (h + 1) * HEAD_STRIDE
                            ]
                            mm(
                                op[:, 256 * hh : 256 * (hh + 1)],
                                vsl,
                                pt[:, 256 * r : 256 * (r + 1)],
                                start=(r == 0),
                                stop=(r == 2),
                            )
                        if hh == 1:
                            otmp = pers.tile(
                                [DH + 1, 512], f32, name=f"otm{t}_{ch}", tag="otmp",
                                bufs=4,
                            )
                            nc.scalar.copy(out=otmp[:], in_=op[:])
                    # decoupled normalization for the head pair
                    rbp = ps.tile([DH, 512], f32, name=f"rbp{t}_{ch}", tag="big", bufs=2)
                    rr = pers.tile([1, 512], f32r, name=f"rr{t}_{ch}", tag="rr", bufs=2)
                    nc.vector.reciprocal(out=rr[:], in_=otmp[DH : DH + 1, :])
                    for hh in range(2):
                        mm(rbp[:, 256 * hh : 256 * (hh + 1)], ones1[:],
                           rr[:, 256 * hh : 256 * (hh + 1)], start=True, stop=True)
                    otc = pers.tile(
                        [128, 256], f32r, name=f"OTc{t}_{ch}", tag=f"OTc{t}", bufs=2
                    )
                    for hh in range(2):
                        nc.vector.tensor_mul(
                            out=otc[64 * hh : 64 * (hh + 1), :],
                            in0=otmp[0:DH, 256 * hh : 256 * (hh + 1)],
                            in1=rbp[:, 256 * hh : 256 * (hh + 1)],
                        )
                    cur_otc.append(otc)
                prev_otc = cur_otc
            emit_D(NCH - 1, prev_otc)

    nc.compile()
    return nc


def get_nc():
    if "nc" not in _NC_CACHE:
        _NC_CACHE["nc"] = _build_nc()
    return _NC_CACHE["nc"]


def make_core_inputs(x, Wqkv, Wout, bout):
    """Host-side shard + layout prep: per-core transposed views."""
    x = np.asarray(x, dtype=np.float32)
    wqkvT = np.ascontiguousarray(np.asarray(Wqkv, dtype=np.float32).T)
    woutT = np.ascontiguousarray(np.asarray(Wout, dtype=np.float32).T)
    bout = np.ascontiguousarray(np.asarray(bout, dtype=np.float32))
    in_maps = []
    for b in range(B):
        in_maps.append(
            {
                "xT": np.ascontiguousarray(x[:, b, :].T),
                "wqkvT": wqkvT,
                "woutT": woutT,
                "bout": bout,
                "zeros_c": np.zeros((128, 640), dtype=np.float32),
                "ones_c": np.ones((128, 8), dtype=np.float32),
            }
        )
    return in_maps


def kernel(x, Wqkv, Wout, bout):
    from concourse.bass_utils import run_bass_kernel_spmd

    nc = get_nc()
    in_maps = make_core_inputs(x, Wqkv, Wout, bout)
    res = run_bass_kernel_spmd(nc, in_maps, core_ids=list(range(B)))
    out = np.empty((L, B, D), dtype=np.float32)
    for b in range(B):
        out[:, b, :] = res.results[b]["y"]
    return out



# revision 16
# speedup vs baseline: 1.2015x; 1.2015x over previous
"""Local-window (banded) multi-head attention on 8 Trainium2 NeuronCores.

Problem: x[L=2048, B=8, D=512], Wqkv[1536, 512], Wout[512, 512], bout[512].
  qkv = x @ Wqkv.T ; per-head banded attention (|i-j| <= 64, window 129);
  out = attn_out @ Wout.T + bout.

Sharding: batch B=8 across the 8 cores (data parallel).

Per-core structure (all matmuls contract over the partition dim):
 - Q/K projection in fp8e4m3 DoubleRow (weights pre-scaled x16 host-side;
   compensated in the exp scale 2^-11). q/k noise is attenuated ~10x through
   the softmax, so fp8 is safe here.
 - V projection in bf16, written directly into 64-row-shifted tiles (17
   half-overlapping tiles) so the banded PV needs no re-blocking copies.
 - Banded scores computed transposed (scoresT[m, l]) per 128-query chunk
   over a 256-key window: 2 m-tiles, bf16. Band masking is done on the PE:
   an fp8e5 DoubleRow matmul adds a constant -57344 upper/lower-triangular
   matrix into the score PSUM (exp then underflows to exactly 0).
 - exp on the scalar engine -> P in bf16 (P in fp8 would cost ~2e-2 rel
   error; bf16 keeps it at ~1e-2 total).
 - PV in bf16 with head pairs STACKED on partitions (out offsets 0/64),
   so normalization is elementwise. Denominators come from fp8 DoubleRow
   ones-matmuls against an fp8 copy of P (sum averaging kills the fp8
   noise), duplicated across 64 rows so the divide needs no broadcast.
 - Output projection in bf16; bias-add + store as bf16.
"""

import sys

import numpy as np
import ml_dtypes

if "/opt/trn_rl_repo" not in sys.path:
    sys.path.insert(0, "/opt/trn_rl_repo")

L, B, D, H, DH = 2048, 8, 512, 8, 64
NCH = L // 128  # 16 attention chunks
NVT = 17  # shifted V tiles
NEG = -57344.0  # e5m2-exact mask value; exp(2^-11 * -57344) == 0 in bf16
WS = 16.0  # host-side Q/K weight scale (keeps fp8 out of subnormals)

_NC_CACHE = {}


def _build_nc():
    from concourse import bacc, mybir, tile

    f32 = mybir.dt.float32
    f16 = mybir.dt.float16
    bf16 = mybir.dt.bfloat16
    f8 = mybir.dt.float8e4
    e5 = mybir.dt.float8e5
    Exp = mybir.ActivationFunctionType.Exp
    DR = mybir.MatmulPerfMode.DoubleRow

    import concourse.bass as bass

    nc = bacc.Bacc(None, target_bir_lowering=False)

    x8_d = nc.dram_tensor("x8", [128, 8192], f8, kind="ExternalInput")
    xb_d = nc.dram_tensor("xb", [128, 8704], bf16, kind="ExternalInput")
    wqk_d = nc.dram_tensor("wqk8", [128, 4096], f8, kind="ExternalInput")
    wv_d = nc.dram_tensor("wvb", [128, 2048], bf16, kind="ExternalInput")
    wo_d = nc.dram_tensor("wob", [128, 2048], bf16, kind="ExternalInput")
    one8_d = nc.dram_tensor("one8", [128, 128], f8, kind="ExternalInput")
    ce5_d = nc.dram_tensor("ce5", [128, 1280], e5, kind="ExternalInput")
    bout_d = nc.dram_tensor("bout", [1, 512], bf16, kind="ExternalInput")
    y_d = nc.dram_tensor("y", [L, D], bf16, kind="ExternalOutput")

    def mm(out, lhsT, rhs, start, stop, pm=None):
        nc.tensor.matmul(out, lhsT, rhs, start=start, stop=stop, perf_mode=pm)

    with tile.TileContext(nc) as tc, nc.allow_low_precision(
        reason="fp8/bf16 tiles feed the PE fast paths; accumulation is fp32"
    ):
        with (
            tc.tile_pool(name="pers", bufs=1) as pers,
            tc.tile_pool(name="ps", bufs=1, space="PSUM") as ps,
        ):
            xt8 = pers.tile([128, 8192], f8, name="xt8", tag="xt8")
            xtb = pers.tile([128, 8704], bf16, name="xtb", tag="xtb")
            wqk = pers.tile([128, 4096], f8, name="wqk", tag="wqk")
            wv = pers.tile([128, 2048], bf16, name="wv", tag="wv")
            wo = pers.tile([128, 2048], bf16, name="wo", tag="wo")
            one8 = pers.tile([128, 128], f8, name="one8", tag="one8")
            ce5 = pers.tile([128, 1280], e5, name="ce5", tag="ce5")
            boutr = pers.tile([1, 512], bf16, name="boutr", tag="boutr")
            onecol = pers.tile([1, 128], bf16, name="onecol", tag="onecol")
            QT = [pers.tile([128, 2048], bf16, name=f"QT{t}", tag=f"QT{t}")
                  for t in range(4)]
            KT = [pers.tile([128, 2176], bf16, name=f"KT{t}", tag=f"KT{t}")
                  for t in range(4)]
            VA = pers.tile([128, NVT * 512], bf16, name="VA", tag="VA")

            # ---- input DMAs, sliced so phase B unblocks early ----
            # Q weights + x8 for lc=0 first, then K weights, V path, rest.
            nc.sync.dma_start(out=wqk[:, 0:2048], in_=wqk_d[:, 0:2048])
            for jj in range(2):
                for ii in range(2):
                    c0 = 4096 * jj + 2048 * ii
                    nc.sync.dma_start(
                        out=xt8[:, c0 : c0 + 512], in_=x8_d[:, c0 : c0 + 512]
                    )
            nc.sync.dma_start(out=wqk[:, 2048:4096], in_=wqk_d[:, 2048:4096])
            nc.scalar.dma_start(out=wv[:], in_=wv_d[:])
            for kt in range(4):
                c0 = 2176 * kt
                nc.scalar.dma_start(
                    out=xtb[:, c0 : c0 + 704], in_=xb_d[:, c0 : c0 + 704]
                )
            nc.scalar.dma_start(out=ce5[:], in_=ce5_d[:])
            nc.scalar.dma_start(out=one8[:], in_=one8_d[:])
            nc.scalar.dma_start(out=boutr[:], in_=bout_d[:])
            nc.vector.memset(onecol[:], 1.0)
            # remaining x slices per lc (interleaved with weight tails)
            for lc in range(1, 4):
                for jj in range(2):
                    for ii in range(2):
                        c0 = 4096 * jj + 2048 * ii + 512 * lc
                        nc.sync.dma_start(
                            out=xt8[:, c0 : c0 + 512], in_=x8_d[:, c0 : c0 + 512]
                        )
                for kt in range(4):
                    c0 = 2176 * kt + 704 + 512 * (lc - 1)
                    w = 448 if lc == 3 else 512
                    nc.scalar.dma_start(
                        out=xtb[:, c0 : c0 + w], in_=xb_d[:, c0 : c0 + w]
                    )
            nc.sync.dma_start(out=wo[:], in_=wo_d[:])
            # KT zero pads (left 64, right 64)
            for t in range(4):
                nc.vector.memset(KT[t][:, 0:64], 0.0)
                nc.vector.memset(KT[t][:, 2112:2176], 0.0)

            ident = ce5[:, 0:256].rearrange("p (i m) -> p i m", i=2)  # [I|Z]

            def utri_rhs(u):
                return ce5[:, 256 + 256 * u : 512 + 256 * u].rearrange(
                    "p (i m) -> p i m", i=2
                )

            onesv = one8[:, 0:128].rearrange("p (i m) -> p i m", i=2)[:, :, 0:64]

            def emit_qk_proj(t, lc, is_k):
                # psum [128ch, 512l] = DR over 2 k-pairs
                wt = t + 4 * is_k
                pj = ps.tile([128, 512], f32, name=f"pj{wt}_{lc}", tag="sc", bufs=2)
                for jj in range(2):
                    lhsT = wqk[:, 512 * wt + 256 * jj : 512 * wt + 256 * jj + 256
                               ].rearrange("p (i m) -> p i m", i=2)
                    rhs = xt8[:, 4096 * jj : 4096 * jj + 4096].rearrange(
                        "p (i l) -> p i l", l=2048
                    )[:, :, 512 * lc : 512 * lc + 512]
                    mm(pj[:], lhsT, rhs, start=(jj == 0), stop=(jj == 1), pm=DR)
                if is_k:
                    dest = KT[t][:, 64 + 512 * lc : 64 + 512 * lc + 512]
                    nc.scalar.copy(out=dest, in_=pj[:])
                else:
                    dest = QT[t][:, 512 * lc : 512 * lc + 512]
                    nc.vector.tensor_copy(out=dest, in_=pj[:])

            def emit_v_proj(j):
                # V tile j covers l in [128j-64, 128j+64); bf16, 4 k-tiles
                vp = ps.tile([128, 512], f32, name=f"vp{j}", tag="sc", bufs=2)
                for kt in range(4):
                    lhsT = xtb[:, 2176 * kt + 128 * j : 2176 * kt + 128 * j + 128]
                    mm(vp[:], lhsT, wv[:, 512 * kt : 512 * kt + 512],
                       start=(kt == 0), stop=(kt == 3))
                # scatter into per-head 64-col blocks: col = 1088h + 64j + e
                dst = VA.rearrange("p (h c) -> p h c", h=H)[
                    :, :, 64 * j : 64 * j + 64
                ]
                src = vp.rearrange("p (h e) -> p h e", e=64)
                nc.vector.tensor_copy(out=dst, in_=src)

            vtiles = [range(0, 5), range(5, 9), range(9, 13), range(13, 17)]

            def emit_chunk(c):
                # scores + bias per pair, exp, P8 cast, PV + dn, divide
                ptiles = []
                p8tiles = []
                for t in range(4):
                    scp = ps.tile([128, 512], f32, name=f"sc{t}_{c}", tag="sc",
                                  bufs=2)
                    for hh in range(2):
                        p0 = 64 * hh
                        qsl = QT[t][p0 : p0 + 64, 128 * c : 128 * c + 128]
                        for r in range(2):
                            out = scp[:, 256 * hh + 128 * r : 256 * hh + 128 * r + 128]
                            ksl = KT[t][p0 : p0 + 64,
                                        128 * c + 128 * r : 128 * c + 128 * r + 128]
                            mm(out, ksl, qsl, start=True, stop=False)
                            u = r
                            if c == 0 and r == 0:
                                u = 2
                            if c == NCH - 1 and r == 1:
                                u = 3
                            mm(out, ident, utri_rhs(u), start=False, stop=True,
                               pm=DR)
                    pt = pers.tile([128, 512], bf16, name=f"pt{t}_{c}", tag="p",
                                   bufs=4)
                    nc.scalar.activation(out=pt[:], in_=scp[:], func=Exp,
                                         scale=0.00048828125)
                    p8 = pers.tile([128, 512], f8, name=f"p8{t}_{c}", tag="p8",
                                   bufs=4)
                    nc.gpsimd.tensor_copy(out=p8[:], in_=pt[:])
                    ptiles.append(pt)
                    p8tiles.append(p8)
                op = ps.tile([128, 512], f32, name=f"op{c}", tag="op", bufs=2)
                dn = ps.tile([64, 1024], f32, name=f"dn{c}", tag="dn", bufs=1)
                for t in range(4):
                    pt, p8 = ptiles[t], p8tiles[t]
                    for hh in range(2):
                        h = 2 * t + hh
                        for r in range(2):
                            vsl = VA[:, 1088 * h + 64 * (c + r) :
                                     1088 * h + 64 * (c + r) + 64]
                            mm(op[64 * hh : 64 * hh + 64, 128 * t : 128 * t + 128],
                               vsl, pt[:, 256 * hh + 128 * r : 256 * hh + 128 * r + 128],
                               start=(r == 0), stop=(r == 1))
                        p8v = p8[:, 256 * hh : 256 * hh + 256].rearrange(
                            "p (two l) -> p two l", two=2
                        )
                        mm(dn[:, 512 * hh + 128 * t : 512 * hh + 128 * t + 128],
                           onesv, p8v, start=True, stop=True, pm=DR)
                rdn = pers.tile([64, 1024], f16, name=f"rdn{c}", tag="rdn",
                                bufs=2)
                nc.vector.reciprocal(out=rdn[:], in_=dn[:])
                ot = pers.tile([128, 512], bf16, name=f"ot{c}", tag="ot", bufs=3)
                nc.vector.tensor_mul(out=ot[0:64, :], in0=op[0:64, :],
                                     in1=rdn[:, 0:512])
                nc.vector.tensor_mul(out=ot[64:128, :], in0=op[64:128, :],
                                     in1=rdn[:, 512:1024])
                return ot

            def emit_outproj(c, ot):
                yp = ps.tile([128, 512], f32, name=f"yp{c}", tag="yp", bufs=2)
                # bias via K=1 ones-matmul, then the 4 bf16 contraction tiles
                mm(yp[:], onecol[:], boutr[:], start=True, stop=False)
                for kt in range(4):
                    mm(yp[:], ot[:, 128 * kt : 128 * kt + 128],
                       wo[:, 512 * kt : 512 * kt + 512],
                       start=False, stop=(kt == 3))
                ysb = pers.tile([128, 512], bf16, name=f"ysb{c}", tag="ysb",
                                bufs=2)
                nc.scalar.copy(out=ysb[:], in_=yp[:])
                nc.sync.dma_start(out=y_d[128 * c : 128 * c + 128, :], in_=ysb[:])

            # chunk c reads keys up to 128c+191, i.e. into l-block c//4 + 1,
            # so attention lags one chunk behind the projections.
            chunk_ranges = [range(0, 3), range(3, 7), range(7, 11), range(11, 16)]
            prev = None
            for lc in range(4):
                for t in range(4):
                    emit_qk_proj(t, lc, is_k=0)
                for t in range(4):
                    emit_qk_proj(t, lc, is_k=1)
                for j in vtiles[lc]:
                    emit_v_proj(j)
                for c in chunk_ranges[lc]:
                    if prev is not None:
                        emit_outproj(c - 1, prev)
                    prev = emit_chunk(c)
            emit_outproj(NCH - 1, prev)

    nc.compile()
    return nc


def get_nc():
    if "nc" not in _NC_CACHE:
        _NC_CACHE["nc"] = _build_nc()
    return _NC_CACHE["nc"]


def make_core_inputs(x, Wqkv, Wout, bout):
    """Host-side shard + layout prep (cheap numpy transposes/casts)."""
    e4, e5, bf = (ml_dtypes.float8_e4m3, ml_dtypes.float8_e5m2,
                  ml_dtypes.bfloat16)
    x = np.asarray(x, dtype=np.float32)
    Wqkv = np.asarray(Wqkv, dtype=np.float32)
    Wout = np.asarray(Wout, dtype=np.float32)
    boutr = np.ascontiguousarray(
        np.asarray(bout, dtype=np.float32).reshape(1, 512).astype(bf)
    )

    # wqk8 [128, 4096]: col = 512t + 256jj + 128ii + m ; 16x scaled
    QK = (Wqkv[0:1024] * WS).astype(e4)  # [c, d]
    wqk8 = np.ascontiguousarray(
        QK.reshape(8, 128, 2, 2, 128).transpose(4, 0, 2, 3, 1).reshape(128, 4096)
    )
    # wvb [128, 2048]: col = 512kt + n
    WV = Wqkv[1024:1536].astype(bf)  # [n, d]
    wvb = np.ascontiguousarray(
        WV.reshape(512, 4, 128).transpose(2, 1, 0).reshape(128, 2048)
    )
    # wob [128, 2048]: col = 512kt + n
    WO = Wout.astype(bf)
    wob = np.ascontiguousarray(
        WO.reshape(512, 4, 128).transpose(2, 1, 0).reshape(128, 2048)
    )
    one8 = np.ones((128, 128), dtype=e4)
    # ce5 [128, 1280]: [I | Z | u0 u0 | u1 u1 | u2 u2 | u3 u3]
    pp, ff = np.mgrid[0:128, 0:128]
    ident = np.eye(128, dtype=np.float32)
    zero = np.zeros((128, 128), np.float32)
    u0 = np.where(pp >= ff, 0.0, NEG)
    u1 = np.where(pp <= ff, 0.0, NEG)
    u2 = np.where(pp < 64, NEG, u0)
    u3 = np.where(pp > 63, NEG, u1)
    ce5 = np.concatenate(
        [ident, zero, u0, u0, u1, u1, u2, u2, u3, u3], axis=1
    ).astype(e5)

    in_maps = []
    for b in range(B):
        xb_ = x[:, b, :]  # [L, D]
        x8 = xb_.astype(e4)  # [l, d]
        # x8 [128, 8192]: col = 2048kb + l -> [p, kb, l] from [l, 128kb+p]
        x8t = np.ascontiguousarray(
            x8.reshape(2048, 4, 128).transpose(2, 1, 0).reshape(128, 8192)
        )
        # xb [128, 8704]: col = 2176kt + (l+64), zero padded
        xpad = np.zeros((2176, 512), np.float32)
        xpad[64:2112] = xb_
        xbt = np.ascontiguousarray(
            xpad.astype(bf).reshape(2176, 4, 128).transpose(2, 1, 0).reshape(
                128, 8704)
        )
        in_maps.append(
            {
                "x8": x8t,
                "xb": xbt,
                "wqk8": wqk8,
                "wvb": wvb,
                "wob": wob,
                "one8": one8,
                "ce5": ce5,
                "bout": boutr,
            }
        )
    return in_maps


def kernel(x, Wqkv, Wout, bout):
    from concourse.bass_utils import run_bass_kernel_spmd

    nc = get_nc()
    in_maps = make_core_inputs(x, Wqkv, Wout, bout)
    res = run_bass_kernel_spmd(nc, in_maps, core_ids=list(range(B)))
    out = np.empty((L, B, D), dtype=np.float32)
    for b in range(B):
        out[:, b, :] = res.results[b]["y"].astype(np.float32)
    return out


# revision 24
# speedup vs baseline: 1.2633x; 1.0514x over previous
"""Local-window (banded) multi-head attention on 8 Trainium2 NeuronCores.

Problem: x[L=2048, B=8, D=512], Wqkv[1536, 512], Wout[512, 512], bout[512].
  qkv = x @ Wqkv.T ; per-head banded attention (|i-j| <= 64, window 129);
  out = attn_out @ Wout.T + bout.

Sharding: batch B=8 across the 8 cores (data parallel).

Per-core structure (all matmuls contract over the partition dim):
 - Q/K projection in fp8e4m3 DoubleRow (weights pre-scaled x16 host-side;
   compensated in the exp scale 2^-11). q/k noise is attenuated ~10x through
   the softmax, so fp8 is safe here.
 - V projection in bf16, written directly into 64-row-shifted tiles (17
   half-overlapping tiles) so the banded PV needs no re-blocking copies.
 - Banded scores computed transposed (scoresT[m, l]) per 128-query chunk
   over a 256-key window: 2 m-tiles, bf16. Band masking is done on the PE:
   an fp8e5 DoubleRow matmul adds a constant -57344 upper/lower-triangular
   matrix into the score PSUM (exp then underflows to exactly 0).
 - exp on the scalar engine -> P in bf16 (P in fp8 would cost ~2e-2 rel
   error; bf16 keeps it at ~1e-2 total).
 - PV in bf16 with head pairs STACKED on partitions (out offsets 0/64),
   so normalization is elementwise. Denominators come from fp8 DoubleRow
   ones-matmuls against an fp8 copy of P (sum averaging kills the fp8
   noise), duplicated across 64 rows so the divide needs no broadcast.
 - Output projection in bf16; bias-add + store as bf16.
"""

import sys

import numpy as np
import ml_dtypes

if "/opt/trn_rl_repo" not in sys.path:
    sys.path.insert(0, "/opt/trn_rl_repo")

L, B, D, H, DH = 2048, 8, 512, 8, 64
NCH = L // 128  # 16 attention chunks
NVT = 17  # shifted V tiles
NEG = -57344.0  # e5m2-exact mask value; exp(2^-11 * -57344) == 0 in bf16
WS = 16.0  # host-side Q/K weight scale (keeps fp8 out of subnormals)

_NC_CACHE = {}


def _build_nc():
    from concourse import bacc, mybir, tile

    f32 = mybir.dt.float32
    f16 = mybir.dt.float16
    bf16 = mybir.dt.bfloat16
    f8 = mybir.dt.float8e4
    e5 = mybir.dt.float8e5
    Exp = mybir.ActivationFunctionType.Exp
    DR = mybir.MatmulPerfMode.DoubleRow

    import concourse.bass as bass

    nc = bacc.Bacc(None, target_bir_lowering=False)

    x8_d = nc.dram_tensor("x8", [128, 8192], f8, kind="ExternalInput")
    xb_d = nc.dram_tensor("xb", [128, 8704], bf16, kind="ExternalInput")
    wqk_d = nc.dram_tensor("wqk8", [128, 4096], f8, kind="ExternalInput")
    wv_d = nc.dram_tensor("wvb", [128, 2048], bf16, kind="ExternalInput")
    wo_d = nc.dram_tensor("wob", [128, 2048], bf16, kind="ExternalInput")
    one8_d = nc.dram_tensor("one8", [128, 128], f8, kind="ExternalInput")
    ce5_d = nc.dram_tensor("ce5", [128, 3328], e5, kind="ExternalInput")
    bout_d = nc.dram_tensor("bout", [1, 512], bf16, kind="ExternalInput")
    y_d = nc.dram_tensor("y", [L, D], bf16, kind="ExternalOutput")

    def mm(out, lhsT, rhs, start, stop, pm=None):
        nc.tensor.matmul(out, lhsT, rhs, start=start, stop=stop, perf_mode=pm)

    with tile.TileContext(nc) as tc, nc.allow_low_precision(
        reason="fp8/bf16 tiles feed the PE fast paths; accumulation is fp32"
    ):
        with (
            tc.tile_pool(name="pers", bufs=1) as pers,
            tc.tile_pool(name="ps", bufs=1, space="PSUM") as ps,
        ):
            xt8 = pers.tile([128, 8192], f8, name="xt8", tag="xt8")
            xtb = pers.tile([128, 8704], bf16, name="xtb", tag="xtb")
            wqk = pers.tile([128, 4096], f8, name="wqk", tag="wqk")
            wv = pers.tile([128, 2048], bf16, name="wv", tag="wv")
            wo = pers.tile([128, 2048], bf16, name="wo", tag="wo")
            one8 = pers.tile([128, 128], f8, name="one8", tag="one8")
            ce5 = pers.tile([128, 3328], e5, name="ce5", tag="ce5")
            boutr = pers.tile([1, 512], bf16, name="boutr", tag="boutr")
            onecol = pers.tile([1, 128], bf16, name="onecol", tag="onecol")
            QT = [pers.tile([128, 2048], bf16, name=f"QT{t}", tag=f"QT{t}")
                  for t in range(4)]
            KT = [pers.tile([128, 2176], bf16, name=f"KT{t}", tag=f"KT{t}")
                  for t in range(4)]
            VA = pers.tile([128, NVT * 512], bf16, name="VA", tag="VA")

            # ---- input DMAs, sliced so phase B unblocks early ----
            # Q weights + x8 for lc=0 first, then K weights, V path, rest.
            nc.sync.dma_start(out=wqk[:, 0:2048], in_=wqk_d[:, 0:2048])
            for jj in range(2):
                for ii in range(2):
                    c0 = 4096 * jj + 2048 * ii
                    nc.sync.dma_start(
                        out=xt8[:, c0 : c0 + 512], in_=x8_d[:, c0 : c0 + 512]
                    )
            nc.sync.dma_start(out=wqk[:, 2048:4096], in_=wqk_d[:, 2048:4096])
            nc.scalar.dma_start(out=wv[:], in_=wv_d[:])
            for kt in range(4):
                c0 = 2176 * kt
                nc.scalar.dma_start(
                    out=xtb[:, c0 : c0 + 704], in_=xb_d[:, c0 : c0 + 704]
                )
            nc.scalar.dma_start(out=ce5[:], in_=ce5_d[:])
            nc.scalar.dma_start(out=one8[:], in_=one8_d[:])
            nc.scalar.dma_start(out=boutr[:], in_=bout_d[:])
            nc.vector.memset(onecol[:], 1.0)
            # remaining x slices per lc (interleaved with weight tails)
            for lc in range(1, 4):
                for jj in range(2):
                    for ii in range(2):
                        c0 = 4096 * jj + 2048 * ii + 512 * lc
                        nc.sync.dma_start(
                            out=xt8[:, c0 : c0 + 512], in_=x8_d[:, c0 : c0 + 512]
                        )
                for kt in range(4):
                    c0 = 2176 * kt + 704 + 512 * (lc - 1)
                    w = 448 if lc == 3 else 512
                    nc.scalar.dma_start(
                        out=xtb[:, c0 : c0 + w], in_=xb_d[:, c0 : c0 + w]
                    )
            nc.sync.dma_start(out=wo[:], in_=wo_d[:])
            # KT zero pads (left 64, right 64)
            for t in range(4):
                nc.vector.memset(KT[t][:, 0:64], 0.0)
                nc.vector.memset(KT[t][:, 2112:2176], 0.0)

            ident = ce5[:, 0:256].rearrange("p (i m) -> p i m", i=2)  # [I|Z]

            def utri_rhs(u):
                # [u_r0h0 | u_r1h0 | u_r0h1 | u_r1h1] then 512 zeros
                return ce5[:, 256 + 1024 * u : 256 + 1024 * u + 1024].rearrange(
                    "p (i m) -> p i m", i=2
                )

            onesv = one8[:, 0:128].rearrange("p (i m) -> p i m", i=2)[:, :, 0:64]

            def emit_qk_proj(t, lc, is_k):
                # psum [128ch, 512l] = DR over 2 k-pairs
                wt = t + 4 * is_k
                pj = ps.tile([128, 512], f32, name=f"pj{wt}_{lc}", tag="sc", bufs=2)
                for jj in range(2):
                    lhsT = wqk[:, 512 * wt + 256 * jj : 512 * wt + 256 * jj + 256
                               ].rearrange("p (i m) -> p i m", i=2)
                    rhs = xt8[:, 4096 * jj : 4096 * jj + 4096].rearrange(
                        "p (i l) -> p i l", l=2048
                    )[:, :, 512 * lc : 512 * lc + 512]
                    mm(pj[:], lhsT, rhs, start=(jj == 0), stop=(jj == 1), pm=DR)
                if is_k:
                    dest = KT[t][:, 64 + 512 * lc : 64 + 512 * lc + 512]
                    nc.scalar.copy(out=dest, in_=pj[:])
                else:
                    dest = QT[t][:, 512 * lc : 512 * lc + 512]
                    nc.vector.tensor_copy(out=dest, in_=pj[:])

            def emit_v_proj(j):
                # V tile j covers l in [128j-64, 128j+64); bf16, 4 k-tiles
                vp = ps.tile([128, 512], f32, name=f"vp{j}", tag="sc", bufs=2)
                for kt in range(4):
                    lhsT = xtb[:, 2176 * kt + 128 * j : 2176 * kt + 128 * j + 128]
                    mm(vp[:], lhsT, wv[:, 512 * kt : 512 * kt + 512],
                       start=(kt == 0), stop=(kt == 3))
                # scatter into per-head 64-col blocks: col = 1088h + 64j + e
                dst = VA.rearrange("p (h c) -> p h c", h=H)[
                    :, :, 64 * j : 64 * j + 64
                ]
                src = vp.rearrange("p (h e) -> p h e", e=64)
                nc.vector.tensor_copy(out=dst, in_=src)

            vtiles = [range(0, 5), range(5, 9), range(9, 13), range(13, 17)]

            def emit_scores(c):
                # scores + one merged bias matmul per pair; exp -> P (bf16);
                # gpsimd cast -> P8 (fp8, denominator feed)
                ptiles = []
                p8tiles = []
                u = 0
                if c == 0:
                    u = 1
                if c == NCH - 1:
                    u = 2
                for t in range(4):
                    scp = ps.tile([128, 512], f32, name=f"sc{t}_{c}", tag="sc",
                                  bufs=2)
                    for hh in range(2):
                        p0 = 64 * hh
                        qsl = QT[t][p0 : p0 + 64, 128 * c : 128 * c + 128]
                        for r in range(2):
                            out = scp[:, 256 * hh + 128 * r : 256 * hh + 128 * r + 128]
                            ksl = KT[t][p0 : p0 + 64,
                                        128 * c + 128 * r : 128 * c + 128 * r + 128]
                            mm(out, ksl, qsl, start=True, stop=False)
                            mm(out, ident,
                               utri_rhs(u)[:, :, 128 * (2 * hh + r) :
                                           128 * (2 * hh + r) + 128],
                               start=False, stop=True, pm=DR)
                    pt = pers.tile([128, 512], bf16, name=f"pt{t}_{c}", tag="p",
                                   bufs=8)
                    nc.scalar.activation(out=pt[:], in_=scp[:], func=Exp,
                                         scale=0.00048828125)
                    p8 = pers.tile([128, 512], f8, name=f"p8{t}_{c}", tag="p8",
                                   bufs=8)
                    nc.gpsimd.tensor_copy(out=p8[:], in_=pt[:])
                    ptiles.append(pt)
                    p8tiles.append(p8)
                return c, ptiles, p8tiles

            def emit_pvdn(pend):
                c, ptiles, p8tiles = pend
                op = ps.tile([128, 512], f32, name=f"op{c}", tag="op", bufs=2)
                dn = ps.tile([64, 1024], f32, name=f"dn{c}", tag="dn", bufs=1)
                for t in range(4):
                    pt, p8 = ptiles[t], p8tiles[t]
                    for hh in range(2):
                        h = 2 * t + hh
                        for r in range(2):
                            vsl = VA[:, 1088 * h + 64 * (c + r) :
                                     1088 * h + 64 * (c + r) + 64]
                            mm(op[64 * hh : 64 * hh + 64, 128 * t : 128 * t + 128],
                               vsl, pt[:, 256 * hh + 128 * r : 256 * hh + 128 * r + 128],
                               start=(r == 0), stop=(r == 1))
                        p8v = p8[:, 256 * hh : 256 * hh + 256].rearrange(
                            "p (two l) -> p two l", two=2
                        )
                        mm(dn[:, 512 * hh + 128 * t : 512 * hh + 128 * t + 128],
                           onesv, p8v, start=True, stop=True, pm=DR)
                rdn = pers.tile([64, 1024], f16, name=f"rdn{c}", tag="rdn",
                                bufs=2)
                nc.vector.reciprocal(out=rdn[:], in_=dn[:])
                ot = pers.tile([128, 512], bf16, name=f"ot{c}", tag="ot", bufs=3)
                nc.vector.tensor_mul(out=ot[0:64, :], in0=op[0:64, :],
                                     in1=rdn[:, 0:512])
                nc.vector.tensor_mul(out=ot[64:128, :], in0=op[64:128, :],
                                     in1=rdn[:, 512:1024])
                return c, ot

            def emit_outproj(c, ot):
                yp = ps.tile([128, 512], f32, name=f"yp{c}", tag="yp", bufs=2)
                # bias via K=1 ones-matmul, then the 4 bf16 contraction tiles
                mm(yp[:], onecol[:], boutr[:], start=True, stop=False)
                for kt in range(4):
                    mm(yp[:], ot[:, 128 * kt : 128 * kt + 128],
                       wo[:, 512 * kt : 512 * kt + 512],
                       start=False, stop=(kt == 3))
                ysb = pers.tile([128, 512], bf16, name=f"ysb{c}", tag="ysb",
                                bufs=2)
                nc.scalar.copy(out=ysb[:], in_=yp[:])
                nc.sync.dma_start(out=y_d[128 * c : 128 * c + 128, :], in_=ysb[:])

            # chunk c reads keys up to 128c+191, i.e. into l-block c//4 + 1,
            # so attention lags one chunk behind the projections. The chunk
            # stages are software-pipelined (scores(c) | pv+norm(c-1) |
            # outproj(c-2)) so no engine head-of-line blocks on another.
            chunk_ranges = [range(0, 3), range(3, 7), range(7, 11), range(11, 16)]
            pend_sc = None
            pend_oj = None
            for lc in range(4):
                for t in range(4):
                    emit_qk_proj(t, lc, is_k=0)
                for t in range(4):
                    emit_qk_proj(t, lc, is_k=1)
                for j in vtiles[lc]:
                    emit_v_proj(j)
                for c in chunk_ranges[lc]:
                    sc = emit_scores(c)
                    if pend_sc is not None:
                        if pend_oj is not None:
                            emit_outproj(*pend_oj)
                        pend_oj = emit_pvdn(pend_sc)
                    pend_sc = sc
            if pend_oj is not None:
                emit_outproj(*pend_oj)
            pend_oj = emit_pvdn(pend_sc)
            emit_outproj(*pend_oj)

    nc.compile()
    return nc


def get_nc():
    if "nc" not in _NC_CACHE:
        _NC_CACHE["nc"] = _build_nc()
    return _NC_CACHE["nc"]


def make_core_inputs(x, Wqkv, Wout, bout):
    """Host-side shard + layout prep (cheap numpy transposes/casts)."""
    e4, e5, bf = (ml_dtypes.float8_e4m3, ml_dtypes.float8_e5m2,
                  ml_dtypes.bfloat16)
    x = np.asarray(x, dtype=np.float32)
    Wqkv = np.asarray(Wqkv, dtype=np.float32)
    Wout = np.asarray(Wout, dtype=np.float32)
    boutr = np.ascontiguousarray(
        np.asarray(bout, dtype=np.float32).reshape(1, 512).astype(bf)
    )

    # wqk8 [128, 4096]: col = 512t + 256jj + 128ii + m ; 16x scaled
    QK = (Wqkv[0:1024] * WS).astype(e4)  # [c, d]
    wqk8 = np.ascontiguousarray(
        QK.reshape(8, 128, 2, 2, 128).transpose(4, 0, 2, 3, 1).reshape(128, 4096)
    )
    # wvb [128, 2048]: col = 512kt + n
    WV = Wqkv[1024:1536].astype(bf)  # [n, d]
    wvb = np.ascontiguousarray(
        WV.reshape(512, 4, 128).transpose(2, 1, 0).reshape(128, 2048)
    )
    # wob [128, 2048]: col = 512kt + n
    WO = Wout.astype(bf)
    wob = np.ascontiguousarray(
        WO.reshape(512, 4, 128).transpose(2, 1, 0).reshape(128, 2048)
    )
    one8 = np.ones((128, 128), dtype=e4)
    # ce5 [128, 3328]: [I | Z] + 3 bias variants [ua ub ua ub | Z Z Z Z]
    pp, ff = np.mgrid[0:128, 0:128]
    ident = np.eye(128, dtype=np.float32)
    zero = np.zeros((128, 128), np.float32)
    u0 = np.where(pp >= ff, 0.0, NEG)
    u1 = np.where(pp <= ff, 0.0, NEG)
    u2 = np.where(pp < 64, NEG, u0)
    u3 = np.where(pp > 63, NEG, u1)
    z4 = np.concatenate([zero] * 4, axis=1)
    ce5 = np.concatenate(
        [ident, zero,
         u0, u1, u0, u1, z4,
         u2, u1, u2, u1, z4,
         u0, u3, u0, u3, z4], axis=1
    ).astype(e5)

    in_maps = []
    for b in range(B):
        xb_ = x[:, b, :]  # [L, D]
        x8 = xb_.astype(e4)  # [l, d]
        # x8 [128, 8192]: col = 2048kb + l -> [p, kb, l] from [l, 128kb+p]
        x8t = np.ascontiguousarray(
            x8.reshape(2048, 4, 128).transpose(2, 1, 0).reshape(128, 8192)
        )
        # xb [128, 8704]: col = 2176kt + (l+64), zero padded
        xpad = np.zeros((2176, 512), np.float32)
        xpad[64:2112] = xb_
        xbt = np.ascontiguousarray(
            xpad.astype(bf).reshape(2176, 4, 128).transpose(2, 1, 0).reshape(
                128, 8704)
        )
        in_maps.append(
            {
                "x8": x8t,
                "xb": xbt,
                "wqk8": wqk8,
                "wvb": wvb,
                "wob": wob,
                "one8": one8,
                "ce5": ce5,
                "bout": boutr,
            }
        )
    return in_maps


def kernel(x, Wqkv, Wout, bout):
    from concourse.bass_utils import run_bass_kernel_spmd

    nc = get_nc()
    in_maps = make_core_inputs(x, Wqkv, Wout, bout)
    res = run_bass_kernel_spmd(nc, in_maps, core_ids=list(range(B)))
    out = np.empty((L, B, D), dtype=np.float32)
    for b in range(B):
        out[:, b, :] = res.results[b]["y"].astype(np.float32)
    return out
